# revision 25
# baseline (speedup 1.0000x reference)
"""DeepSeekMoE Trainium2 kernel.

Strategy: data-parallel over tokens. Each of the 8 NeuronCores gets 1024 of
the 8192 tokens and computes routing + shared expert + all 8 routed experts
densely (unrouted experts contribute with combine weight 0, matching the
dense reference formulation exactly).

Layout: feature-major ("X^T") activations [d, tokens] so every weight matrix
is used in its natural [in, out] layout as the matmul stationary operand.

Numerics: expert MLPs run in bf16 (fp32 PSUM accumulation). Routing logits
are computed with a bf16 hi/lo split (X = hi + lo), giving ~fp32-accuracy
logits. Top-2 selection runs on the raw logits (sqrt(softplus) is monotonic,
so the selection is identical to the reference's); the combine weights are
then sqrt(ln(1+exp(logits)))*mask via the ACT exp/ln/sqrt tables.

Built with bacc.Bacc and nc.compile(), which legalizes semaphore waits
(1 wait/instruction on TRN2, split via EventSemaphores) and inserts the
ACT function-table loads (exp/ln -> sqrt -> silu, 2 switches). Weight
loads are 4-way split and prefetched one expert ahead; X slices are
independent tiles so consumers wait per-slice; outputs stream out as
their final accumulation lands.
"""

import numpy as np
import ml_dtypes

import concourse.bass as bass
import concourse.mybir as mybir
import concourse.tile as tile
from concourse import bacc
from concourse.tile_rust import add_dep_helper
from concourse.bass_utils import run_bass_kernel_spmd

BF16 = mybir.dt.bfloat16
F32 = mybir.dt.float32
AF = mybir.ActivationFunctionType

N_CORES = 8
B, T, D = 4, 2048, 1024
E = 8            # routed experts
H = 512          # expert hidden dim
NT = (B * T) // N_CORES   # tokens per core = 1024
DT = D // 128    # 8 d-tiles
HT = H // 128    # 4 h-tiles
NE = E + 1       # experts + shared (index 8 = shared)
CT = 512         # psum token chunk
NCH = NT // CT   # 2 chunks
WCOLS = (DT * HT + HT * DT) * 128  # 8192 weight columns per expert


def _build_bass():
    nc = bacc.Bacc("TRN2", num_devices=N_CORES, target_bir_lowering=False)

    xhi = nc.dram_tensor("xhi", [128, DT, NT], BF16, kind="ExternalInput")
    xlo = nc.dram_tensor("xlo", [128, DT, NT], BF16, kind="ExternalInput")
    rwh = nc.dram_tensor("rwh", [128, DT, E], BF16, kind="ExternalInput")
    rwl = nc.dram_tensor("rwl", [128, DT, E], BF16, kind="ExternalInput")
    ebias = nc.dram_tensor("ebias", [1, E], F32, kind="ExternalInput")
    wall = nc.dram_tensor("wall", [NE, 128, WCOLS], BF16, kind="ExternalInput")
    b1h = nc.dram_tensor("b1h", [128, NE, HT], F32, kind="ExternalInput")
    b2h = nc.dram_tensor("b2h", [128, DT], F32, kind="ExternalInput")
    rb2h = nc.dram_tensor("rb2h", [E, DT, 128], BF16, kind="ExternalInput")
    yt = nc.dram_tensor("yt", [128, DT, NT], F32, kind="ExternalOutput")

    all_dmas = []          # every dma_start instruction, in order
    last_on = {}           # engine name -> last instruction

    def dma(engine, out, in_):
        inst = engine.dma_start(out=out, in_=in_)
        all_dmas.append(inst)
        return inst

    def tr(name, inst):
        last_on[name] = inst
        return inst

    with tile.TileContext(nc) as tc:
        with (
            tc.tile_pool(name="singles", bufs=1) as singles,
            tc.tile_pool(name="wpool", bufs=2) as wpool,
            tc.tile_pool(name="rt", bufs=4) as rt,
            tc.tile_pool(name="h1p", bufs=2) as h1p,
            tc.tile_pool(name="yp", bufs=1) as yp,
            tc.tile_pool(name="ps", bufs=1, space="PSUM") as ps,
            tc.tile_pool(name="dram", bufs=1, space="DRAM") as dram,
        ):
            # ---- static loads -------------------------------------------------
            # small constants first (cheap SP issue slots), then X slices as
            # independent tiles so consumers wait per-slice, with the first
            # expert's weights interleaved before xlo.
            rwh_sb = singles.tile([128, DT, E], BF16)
            dma(nc.sync, rwh_sb, rwh[:, :, :])
            rwl_sb = singles.tile([128, DT, E], BF16)
            dma(nc.sync, rwl_sb, rwl[:, :, :])
            eb_sb = singles.tile([E, 1], F32)
            dma(nc.sync, eb_sb, bass.AP(ebias, 0, [[1, E], [0, 1]]))
            b1_sb = singles.tile([128, NE, HT], F32)
            dma(nc.sync, b1_sb, b1h[:, :, :])
            b2_sb = singles.tile([128, DT], F32)
            dma(nc.sync, b2_sb, b2h[:, :])
            rb2_sb = singles.tile([E, DT, 128], BF16)
            dma(nc.sync, rb2_sb, rb2h[:, :, :])

            xhi_t = []
            for k in range(DT):
                xhi_k = singles.tile([128, NT], BF16, tag=f"xhi{k}")
                # two half-DMAs ride two HWDGE queues -> ~2x faster arrival
                dma(nc.sync, xhi_k[:, 0:CT], xhi[:, k, 0:CT])
                dma(nc.sync, xhi_k[:, CT:NT], xhi[:, k, CT:NT])
                xhi_t.append(xhi_k)

            xlo_t = []
            for k in range(DT):
                xlo_k = singles.tile([128, NT], BF16, tag=f"xlo{k}")
                dma(nc.sync, xlo_k[:, 0:CT], xlo[:, k, 0:CT])
                dma(nc.sync, xlo_k[:, CT:NT], xlo[:, k, CT:NT])
                xlo_t.append(xlo_k)

            wdmas = []

            def load_w(e):
                tiles = []
                for q in range(4):
                    w_q = wpool.tile([128, WCOLS // 4], BF16, tag=f"w{q}", bufs=2)
                    tiles.append(w_q)
                    wdmas.append(dma(nc.sync, w_q,
                                     wall[e, :, bass.ds(q * (WCOLS // 4), WCOLS // 4)]))
                return tiles

            w_next = load_w(E)  # shared expert's weights prefetch

            ident16 = singles.tile([128, 128], BF16)
            nc.gpsimd.memset(ident16, 0.0)
            tr("pool", nc.gpsimd.affine_select(
                out=ident16, in_=ident16, compare_op=mybir.AluOpType.not_equal,
                fill=1.0, base=0, pattern=[[-1, 128]], channel_multiplier=1))
            ident8b = singles.tile([8, 8], BF16)
            nc.gpsimd.memset(ident8b, 0.0)
            tr("pool", nc.gpsimd.affine_select(
                out=ident8b, in_=ident8b, compare_op=mybir.AluOpType.not_equal,
                fill=1.0, base=0, pattern=[[-1, 8]], channel_multiplier=1))

            # warm-up: have DVE observe the small constant tiles' DMA queues
            # so later 1-wait-capped TensorScalarPtr ops don't need them
            warm = singles.tile([128, 8], F32)
            tr("dve", nc.vector.tensor_copy(warm[0:E, 0:1], eb_sb))
            tr("dve", nc.vector.tensor_copy(warm[:, 1:2], b2_sb[:, 0:1]))
            tr("dve", nc.vector.tensor_copy(warm[:, 2:3], b1_sb[:, 0, 0:1]))

            combine_sb = singles.tile([E, NT], F32)
            combine16_sb = singles.tile([E, NT], BF16)
            mask_sb = singles.tile([E, NT], BF16)
            cbc_sb = singles.tile([128, E, NT], F32)
            comb_dram = dram.tile([E, NT], F32)

            # ---- routing ------------------------------------------------------
            # logits feature-major [E, t] = sum_k rwt[dk].T @ Xt[dk]; bf16 hi/lo
            sp_tiles = []
            lg_tiles = []
            for c in range(NCH):
                lg_c = ps.tile([E, CT], F32, tag="ph", bufs=4)
                lg_tiles.append(lg_c)
            # product-major emission: both chunks' hi-terms first so the PE
            # doesn't stall mid-group waiting for the xlo DMAs
            prev_mm = None
            for pi, (xa, wa) in enumerate(
                    ((xhi_t, rwh_sb), (xhi_t, rwl_sb), (xlo_t, rwh_sb))):
                for c in range(NCH):
                    for k in range(DT):
                        mm = tr("pe", nc.tensor.matmul(
                            lg_tiles[c][:, :],
                            wa[:, k, :],
                            xa[k][:, bass.ds(c * CT, CT)],
                            start=(pi == 0 and k == 0),
                            stop=(pi == 2 and k == DT - 1),
                            skip_group_check=True,
                        ))
                        if prev_mm is not None:
                            # keep product-major order through the scheduler so
                            # the xlo-dependent terms issue last
                            add_dep_helper(mm.ins, prev_mm.ins, False, "mm order")
                        prev_mm = mm
            for c in range(NCH):
                tsl = bass.ds(c * CT, CT)
                lg_ps = lg_tiles[c]
                # logits + expert_bias -> sbuf (fp32)
                lgf = rt.tile([E, CT], F32, tag="lgf", bufs=2)
                tr("dve", nc.vector.tensor_scalar(
                    lgf, lg_ps, eb_sb[:, :], None, op0=mybir.AluOpType.add))
                # hi/lo split for exact bf16 transposes
                lghi = rt.tile([E, CT], BF16, tag="lghi", bufs=2)
                tr("dve", nc.vector.tensor_copy(lghi, lgf))
                lglo32 = rt.tile([E, CT], F32, tag="lglo32", bufs=2)
                tr("dve", nc.vector.tensor_sub(lglo32, lgf, lghi))
                lglo = rt.tile([E, CT], BF16, tag="lglo", bufs=2)
                tr("dve", nc.vector.tensor_copy(lglo, lglo32))
                # softplus (exp then ln) on ACT, feature-major
                u_t = rt.tile([E, CT], F32, tag="u", bufs=2)
                tr("act", nc.scalar.activation(u_t, lgf, AF.Exp))
                sp_t = rt.tile([E, CT], F32, tag="sp", bufs=2)
                tr("act", nc.scalar.activation(sp_t, u_t, AF.Ln, bias=1.0, scale=1.0))
                sp_tiles.append(sp_t)

                for s4 in range(CT // 128):
                    s = c * (CT // 128) + s4
                    csl = bass.ds(s4 * 128, 128)
                    trh_ps = ps.tile([128, E], BF16, tag="py", bufs=4)
                    tr("pe", nc.tensor.matmul(trh_ps[:, :], lghi[:, csl], ident8b[:, :],
                                              is_transpose=True, start=True, stop=True))
                    trl_ps = ps.tile([128, E], BF16, tag="py", bufs=4)
                    tr("pe", nc.tensor.matmul(trl_ps[:, :], lglo[:, csl], ident8b[:, :],
                                              is_transpose=True, start=True, stop=True))
                    trh_sb = rt.tile([128, E], F32, tag="trh")
                    tr("dve", nc.vector.tensor_copy(trh_sb, trh_ps))
                    lg_tok = rt.tile([128, E], F32, tag="lgtok")
                    tr("dve", nc.vector.tensor_add(lg_tok, trl_ps, trh_sb))
                    # top-2 selection on logits (monotonic in the activation)
                    m1 = rt.tile([128, 1], F32, tag="m1")
                    tr("dve", nc.vector.tensor_reduce(m1, lg_tok, axis=mybir.AxisListType.X, op=mybir.AluOpType.max))
                    msk = rt.tile([128, E], F32, tag="msk")
                    tr("dve", nc.vector.tensor_scalar(msk, lg_tok, m1[:, :], None, op0=mybir.AluOpType.is_ge))
                    masked = rt.tile([128, E], F32, tag="mskd")
                    tr("dve", nc.vector.scalar_tensor_tensor(
                        masked, msk, -1e30, lg_tok,
                        op0=mybir.AluOpType.mult, op1=mybir.AluOpType.add))
                    m2 = rt.tile([128, 1], F32, tag="m2")
                    tr("dve", nc.vector.tensor_reduce(m2, masked, axis=mybir.AxisListType.X, op=mybir.AluOpType.max))
                    mask01 = rt.tile([128, E], BF16, tag="mask01")
                    tr("dve", nc.vector.tensor_scalar(mask01, lg_tok, m2[:, :], None, op0=mybir.AluOpType.is_ge))
                    # mask back to feature-major (0/1 exact in bf16)
                    mT_ps = ps.tile([E, 128], BF16, tag="py", bufs=4)
                    tr("pe", nc.tensor.matmul(mT_ps[:, :], mask01[:, :], ident16[:, :],
                                              is_transpose=True, start=True, stop=True))
                    tr("dve", nc.vector.tensor_copy(mask_sb[:, bass.ds(s * 128, 128)], mT_ps))

            # combine = sqrt(softplus * mask); sqrt last so the ACT table
            # order is exp/ln -> sqrt -> silu (2 switches total)
            for c in range(NCH):
                tsl = bass.ds(c * CT, CT)
                spm = rt.tile([E, CT], F32, tag="spm", bufs=2)
                tr("dve", nc.vector.tensor_mul(spm, sp_tiles[c], mask_sb[:, tsl]))
                tr("act", nc.scalar.activation(combine_sb[:, tsl], spm, AF.Sqrt))
                tr("dve", nc.vector.tensor_copy(combine16_sb[:, tsl], combine_sb[:, tsl]))

            # bounce combine through DRAM to broadcast rows across partitions
            dma(nc.sync, comb_dram[:, :], combine_sb[:, :])
            for e in range(E):
                dma(nc.sync, cbc_sb[:, e, :],
                    bass.AP(comb_dram.tensor, comb_dram.offset + e * NT, [[0, 128], [1, NT]]))

            # ---- expert MLPs --------------------------------------------------
            # y accumulated in SBUF across experts; shared expert (8) first.
            y_acc = []
            for c in range(NCH):
                y_acc_c = yp.tile([128, DT, CT], F32, tag=f"yacc{c}", bufs=1)
                y_acc.append(y_acc_c)
            eorder = [E] + list(range(E))
            for ei, e in enumerate(eorder):
                WQ = WCOLS // 4
                w_tiles = w_next
                if ei + 1 < NE:
                    w_next = load_w(eorder[ei + 1])
                last_e = ei == NE - 1

                def wcol(col):
                    return w_tiles[col // WQ][:, bass.ds(col % WQ, 128)]

                def w1t(k, j):
                    return wcol((k * HT + j) * 128)

                def w2t(j, d):
                    return wcol((DT * HT + j * DT + d) * 128)

                h1_tiles = {}
                for j in range(HT):
                    for c in range(NCH):
                        tsl = bass.ds(c * CT, CT)
                        h1 = h1p.tile([128, CT], BF16, tag=f"h1_{j}_{c}", bufs=3)
                        h1_tiles[(j, c)] = h1
                        ph = ps.tile([128, CT], F32, tag="ph", bufs=4)
                        for k in range(DT):
                            tr("pe", nc.tensor.matmul(
                                ph[:, :],
                                w1t(k, j),
                                xhi_t[k][:, tsl],
                                start=(k == 0), stop=(k == DT - 1),
                            ))
                        tr("act", nc.scalar.activation(
                            h1[:, :], ph, AF.Silu,
                            bias=b1_sb[:, e, j:j + 1], scale=1.0))
                        if e != E:
                            # in-place combine scale (keeps DVE as last writer)
                            tr("dve", nc.vector.tensor_mul(h1, h1, cbc_sb[:, e, tsl]))

                for c in range(NCH):
                    tsl = bass.ds(c * CT, CT)
                    for d in range(DT):
                        py = ps.tile([128, CT], F32, tag="py", bufs=4)
                        for j in range(HT):
                            tr("pe", nc.tensor.matmul(
                                py[:, :],
                                w2t(j, d),
                                h1_tiles[(j, c)][:, :],
                                start=(j == 0),
                                stop=(j == HT - 1 and not last_e),
                            ))
                        if last_e:
                            # rb2^T @ combine joins the last expert's group
                            tr("pe", nc.tensor.matmul(
                                py[:, :], rb2_sb[:, d, :], combine16_sb[:, tsl],
                                start=False, stop=True))
                        if ei == 0:
                            # shared expert first: y = py + sb2
                            tr("dve", nc.vector.tensor_scalar(
                                y_acc[c][:, d, :], py[:, :], b2_sb[:, d:d + 1], None,
                                op0=mybir.AluOpType.add))
                        else:
                            tr("dve", nc.vector.tensor_add(
                                y_acc[c][:, d, :], y_acc[c][:, d, :], py[:, :]))
                        if last_e:
                            # stream each output slice out as soon as its
                            # final accumulation lands
                            dma(nc.sync, yt[:, d, tsl], y_acc[c][:, d, :])

    nc.compile()
    return nc


_NC_CACHE = None


def _get_bass():
    global _NC_CACHE
    if _NC_CACHE is None:
        _NC_CACHE = _build_bass()
    return _NC_CACHE


def _pack_inputs(X, routing_W, expert_bias, sW1, sb1, sW2, sb2, rW1, rb1, rW2, rb2):
    bf = ml_dtypes.bfloat16
    f32 = np.float32

    Xf = np.ascontiguousarray(np.asarray(X, f32).reshape(B * T, D))

    w1 = np.concatenate([np.asarray(rW1, f32), np.asarray(sW1, f32)[None]], axis=0)
    w2 = np.concatenate([np.asarray(rW2, f32), np.asarray(sW2, f32)[None]], axis=0)
    b1 = np.concatenate([np.asarray(rb1, f32), np.asarray(sb1, f32)[None]], axis=0)

    w1h = w1.reshape(NE, DT, 128, HT, 128).transpose(0, 2, 1, 3, 4).reshape(
        NE, 128, DT * HT * 128)
    w2h = w2.reshape(NE, HT, 128, DT, 128).transpose(0, 2, 1, 3, 4).reshape(
        NE, 128, HT * DT * 128)
    wall = np.ascontiguousarray(np.concatenate([w1h, w2h], axis=2)).astype(bf)
    b1h = np.ascontiguousarray(b1.reshape(NE, HT, 128).transpose(2, 0, 1)).astype(f32)
    b2h = np.ascontiguousarray(np.asarray(sb2, f32).reshape(DT, 128).T).astype(f32)
    rb2h = np.ascontiguousarray(np.asarray(rb2, f32).reshape(E, DT, 128)).astype(bf)

    rwt = np.ascontiguousarray(np.asarray(routing_W, f32).T)       # [1024, 8]
    rwt_hi = rwt.astype(bf)
    rwt_lo = (rwt - rwt_hi.astype(f32)).astype(bf)
    rwh = np.ascontiguousarray(rwt_hi.reshape(DT, 128, E).transpose(1, 0, 2))
    rwl = np.ascontiguousarray(rwt_lo.reshape(DT, 128, E).transpose(1, 0, 2))

    eb = np.ascontiguousarray(np.asarray(expert_bias, f32).reshape(1, E))

    shared = dict(rwh=rwh, rwl=rwl, ebias=eb, wall=wall,
                  b1h=b1h, b2h=b2h, rb2h=rb2h)

    in_maps = []
    for c in range(N_CORES):
        Xs = np.ascontiguousarray(Xf[c * NT:(c + 1) * NT].T)  # [1024 d, 1024 t] f32
        xhi_a = Xs.astype(bf)
        xlo_a = (Xs - xhi_a.astype(f32)).astype(bf)
        xhi_p = np.ascontiguousarray(xhi_a.reshape(DT, 128, NT).transpose(1, 0, 2))
        xlo_p = np.ascontiguousarray(xlo_a.reshape(DT, 128, NT).transpose(1, 0, 2))
        in_maps.append(dict(xhi=xhi_p, xlo=xlo_p, **shared))
    return in_maps


def kernel(X, routing_W, expert_bias, sW1, sb1, sW2, sb2, rW1, rb1, rW2, rb2,
           _trace=False):
    in_maps = _pack_inputs(X, routing_W, expert_bias, sW1, sb1, sW2, sb2,
                           rW1, rb1, rW2, rb2)
    nc = _get_bass()
    res = run_bass_kernel_spmd(nc, in_maps, core_ids=list(range(N_CORES)),
                               trace=_trace)
    out = np.empty((B * T, D), np.float32)
    for c in range(N_CORES):
        ytc = res.results[c]["yt"]                       # [128, DT, NT]
        Yt = ytc.transpose(1, 0, 2).reshape(D, NT)       # [d, t]
        out[c * NT:(c + 1) * NT] = Yt.T
    out = out.reshape(B, T, D)
    if _trace:
        return out, res
    return out


# revision 26
# speedup vs baseline: 1.0190x; 1.0190x over previous
"""DeepSeekMoE Trainium2 kernel.

Strategy: data-parallel over tokens. Each of the 8 NeuronCores gets 1024 of
the 8192 tokens and computes routing + shared expert + all 8 routed experts
densely (unrouted experts contribute with combine weight 0, matching the
dense reference formulation exactly).

Layout: feature-major ("X^T") activations [d, tokens] so every weight matrix
is used in its natural [in, out] layout as the matmul stationary operand.

Numerics: expert MLPs run in bf16 (fp32 PSUM accumulation). Routing logits
are computed with a bf16 hi/lo split (X = hi + lo), giving ~fp32-accuracy
logits. Top-2 selection runs on the raw logits (sqrt(softplus) is monotonic,
so the selection is identical to the reference's); the combine weights are
then sqrt(ln(1+exp(logits)))*mask via the ACT exp/ln/sqrt tables.

Built with bacc.Bacc and nc.compile(), which legalizes semaphore waits
(1 wait/instruction on TRN2, split via EventSemaphores) and inserts the
ACT function-table loads (exp/ln -> sqrt -> silu, 2 switches). Weight
loads are 4-way split and prefetched one expert ahead; X slices are
independent tiles so consumers wait per-slice; outputs stream out as
their final accumulation lands.
"""

import numpy as np
import ml_dtypes

import concourse.bass as bass
import concourse.mybir as mybir
import concourse.tile as tile
from concourse import bacc
from concourse.tile_rust import add_dep_helper
from concourse.bass_utils import run_bass_kernel_spmd

BF16 = mybir.dt.bfloat16
F32 = mybir.dt.float32
AF = mybir.ActivationFunctionType

N_CORES = 8
B, T, D = 4, 2048, 1024
E = 8            # routed experts
H = 512          # expert hidden dim
NT = (B * T) // N_CORES   # tokens per core = 1024
DT = D // 128    # 8 d-tiles
HT = H // 128    # 4 h-tiles
NE = E + 1       # experts + shared (index 8 = shared)
CT = 512         # psum token chunk
NCH = NT // CT   # 2 chunks
WCOLS = (DT * HT + HT * DT) * 128  # 8192 weight columns per expert


def _build_bass():
    nc = bacc.Bacc("TRN2", num_devices=N_CORES, target_bir_lowering=False)

    xhi = nc.dram_tensor("xhi", [128, DT, NT], BF16, kind="ExternalInput")
    xlo = nc.dram_tensor("xlo", [128, DT, NT], BF16, kind="ExternalInput")
    rwh = nc.dram_tensor("rwh", [128, DT, E], BF16, kind="ExternalInput")
    rwl = nc.dram_tensor("rwl", [128, DT, E], BF16, kind="ExternalInput")
    ebias = nc.dram_tensor("ebias", [1, E], F32, kind="ExternalInput")
    wall = nc.dram_tensor("wall", [NE, 128, WCOLS], BF16, kind="ExternalInput")
    b1h = nc.dram_tensor("b1h", [128, NE, HT], F32, kind="ExternalInput")
    b2h = nc.dram_tensor("b2h", [128, DT], F32, kind="ExternalInput")
    rb2h = nc.dram_tensor("rb2h", [E, DT, 128], BF16, kind="ExternalInput")
    yt = nc.dram_tensor("yt", [128, DT, NT], F32, kind="ExternalOutput")

    all_dmas = []          # every dma_start instruction, in order
    last_on = {}           # engine name -> last instruction

    def dma(engine, out, in_):
        inst = engine.dma_start(out=out, in_=in_)
        all_dmas.append(inst)
        return inst

    def tr(name, inst):
        last_on[name] = inst
        return inst

    with tile.TileContext(nc) as tc:
        with (
            tc.tile_pool(name="singles", bufs=1) as singles,
            tc.tile_pool(name="wpool", bufs=2) as wpool,
            tc.tile_pool(name="rt", bufs=4) as rt,
            tc.tile_pool(name="h1p", bufs=2) as h1p,
            tc.tile_pool(name="yp", bufs=1) as yp,
            tc.tile_pool(name="ps", bufs=1, space="PSUM") as ps,
            tc.tile_pool(name="dram", bufs=1, space="DRAM") as dram,
        ):
            # ---- static loads -------------------------------------------------
            # small constants first (cheap SP issue slots), then X slices as
            # independent tiles so consumers wait per-slice, with the first
            # expert's weights interleaved before xlo.
            rwh_sb = singles.tile([128, DT, E], BF16)
            dma(nc.sync, rwh_sb, rwh[:, :, :])
            rwl_sb = singles.tile([128, DT, E], BF16)
            dma(nc.sync, rwl_sb, rwl[:, :, :])
            eb_sb = singles.tile([E, 1], F32)
            dma(nc.sync, eb_sb, bass.AP(ebias, 0, [[1, E], [0, 1]]))
            b1_sb = singles.tile([128, NE, HT], F32)
            dma(nc.sync, b1_sb, b1h[:, :, :])
            b2_sb = singles.tile([128, DT], F32)
            dma(nc.sync, b2_sb, b2h[:, :])
            rb2_sb = singles.tile([E, DT, 128], BF16)
            dma(nc.sync, rb2_sb, rb2h[:, :, :])

            xhi_t = []
            for k in range(DT):
                xhi_k = singles.tile([128, NT], BF16, tag=f"xhi{k}")
                dma(nc.sync, xhi_k, xhi[:, k, :])
                xhi_t.append(xhi_k)

            xlo_t = []
            for k in range(DT):
                xlo_k = singles.tile([128, NT], BF16, tag=f"xlo{k}")
                dma(nc.sync, xlo_k, xlo[:, k, :])
                xlo_t.append(xlo_k)

            wdmas = []

            def load_w(e):
                tiles = []
                for q in range(4):
                    w_q = wpool.tile([128, WCOLS // 4], BF16, tag=f"w{q}", bufs=2)
                    tiles.append(w_q)
                    wdmas.append(dma(nc.sync, w_q,
                                     wall[e, :, bass.ds(q * (WCOLS // 4), WCOLS // 4)]))
                return tiles

            w_next = load_w(E)  # shared expert's weights prefetch

            ident16 = singles.tile([128, 128], BF16)
            nc.gpsimd.memset(ident16, 0.0)
            tr("pool", nc.gpsimd.affine_select(
                out=ident16, in_=ident16, compare_op=mybir.AluOpType.not_equal,
                fill=1.0, base=0, pattern=[[-1, 128]], channel_multiplier=1))
            ident8b = singles.tile([8, 8], BF16)
            nc.gpsimd.memset(ident8b, 0.0)
            tr("pool", nc.gpsimd.affine_select(
                out=ident8b, in_=ident8b, compare_op=mybir.AluOpType.not_equal,
                fill=1.0, base=0, pattern=[[-1, 8]], channel_multiplier=1))

            # warm-up: have DVE observe the small constant tiles' DMA queues
            # so later 1-wait-capped TensorScalarPtr ops don't need them
            warm = singles.tile([128, 8], F32)
            tr("dve", nc.vector.tensor_copy(warm[0:E, 0:1], eb_sb))
            tr("dve", nc.vector.tensor_copy(warm[:, 1:2], b2_sb[:, 0:1]))
            tr("dve", nc.vector.tensor_copy(warm[:, 2:3], b1_sb[:, 0, 0:1]))

            combine_sb = singles.tile([E, NT], F32)
            combine16_sb = singles.tile([E, NT], BF16)
            mask_sb = singles.tile([E, NT], BF16)
            cbc_sb = singles.tile([128, E, NT], F32)
            comb_dram = dram.tile([E, NT], F32)

            # ---- routing ------------------------------------------------------
            # logits feature-major [E, t] = sum_k rwt[dk].T @ Xt[dk]; bf16 hi/lo
            sp_tiles = []
            lg_tiles = []
            for c in range(NCH):
                lg_c = ps.tile([E, CT], F32, tag="ph", bufs=4)
                lg_tiles.append(lg_c)
            # product-major emission: both chunks' hi-terms first so the PE
            # doesn't stall mid-group waiting for the xlo DMAs
            prev_mm = None
            for pi, (xa, wa) in enumerate(
                    ((xhi_t, rwh_sb), (xhi_t, rwl_sb), (xlo_t, rwh_sb))):
                for c in range(NCH):
                    for k in range(DT):
                        mm = tr("pe", nc.tensor.matmul(
                            lg_tiles[c][:, :],
                            wa[:, k, :],
                            xa[k][:, bass.ds(c * CT, CT)],
                            start=(pi == 0 and k == 0),
                            stop=(pi == 2 and k == DT - 1),
                            skip_group_check=True,
                        ))
                        if prev_mm is not None:
                            # keep product-major order through the scheduler so
                            # the xlo-dependent terms issue last
                            add_dep_helper(mm.ins, prev_mm.ins, False, "mm order")
                        prev_mm = mm
            for c in range(NCH):
                tsl = bass.ds(c * CT, CT)
                lg_ps = lg_tiles[c]
                # logits + expert_bias -> sbuf (fp32)
                lgf = rt.tile([E, CT], F32, tag="lgf", bufs=2)
                tr("dve", nc.vector.tensor_scalar(
                    lgf, lg_ps, eb_sb[:, :], None, op0=mybir.AluOpType.add))
                # hi/lo split for exact bf16 transposes
                lghi = rt.tile([E, CT], BF16, tag="lghi", bufs=2)
                tr("dve", nc.vector.tensor_copy(lghi, lgf))
                lglo32 = rt.tile([E, CT], F32, tag="lglo32", bufs=2)
                tr("dve", nc.vector.tensor_sub(lglo32, lgf, lghi))
                lglo = rt.tile([E, CT], BF16, tag="lglo", bufs=2)
                tr("dve", nc.vector.tensor_copy(lglo, lglo32))
                # softplus (exp then ln) on ACT, feature-major
                u_t = rt.tile([E, CT], F32, tag="u", bufs=2)
                tr("act", nc.scalar.activation(u_t, lgf, AF.Exp))
                sp_t = rt.tile([E, CT], F32, tag="sp", bufs=2)
                tr("act", nc.scalar.activation(sp_t, u_t, AF.Ln, bias=1.0, scale=1.0))
                sp_tiles.append(sp_t)

                for s4 in range(CT // 128):
                    s = c * (CT // 128) + s4
                    csl = bass.ds(s4 * 128, 128)
                    trh_ps = ps.tile([128, E], BF16, tag="py", bufs=4)
                    tr("pe", nc.tensor.matmul(trh_ps[:, :], lghi[:, csl], ident8b[:, :],
                                              is_transpose=True, start=True, stop=True))
                    trl_ps = ps.tile([128, E], BF16, tag="py", bufs=4)
                    tr("pe", nc.tensor.matmul(trl_ps[:, :], lglo[:, csl], ident8b[:, :],
                                              is_transpose=True, start=True, stop=True))
                    trh_sb = rt.tile([128, E], F32, tag="trh")
                    tr("dve", nc.vector.tensor_copy(trh_sb, trh_ps))
                    lg_tok = rt.tile([128, E], F32, tag="lgtok")
                    tr("dve", nc.vector.tensor_add(lg_tok, trl_ps, trh_sb))
                    # top-2 selection on logits (monotonic in the activation)
                    m1 = rt.tile([128, 1], F32, tag="m1")
                    tr("dve", nc.vector.tensor_reduce(m1, lg_tok, axis=mybir.AxisListType.X, op=mybir.AluOpType.max))
                    msk = rt.tile([128, E], F32, tag="msk")
                    tr("dve", nc.vector.tensor_scalar(msk, lg_tok, m1[:, :], None, op0=mybir.AluOpType.is_ge))
                    masked = rt.tile([128, E], F32, tag="mskd")
                    tr("dve", nc.vector.scalar_tensor_tensor(
                        masked, msk, -1e30, lg_tok,
                        op0=mybir.AluOpType.mult, op1=mybir.AluOpType.add))
                    m2 = rt.tile([128, 1], F32, tag="m2")
                    tr("dve", nc.vector.tensor_reduce(m2, masked, axis=mybir.AxisListType.X, op=mybir.AluOpType.max))
                    mask01 = rt.tile([128, E], BF16, tag="mask01")
                    tr("dve", nc.vector.tensor_scalar(mask01, lg_tok, m2[:, :], None, op0=mybir.AluOpType.is_ge))
                    # mask back to feature-major (0/1 exact in bf16)
                    mT_ps = ps.tile([E, 128], BF16, tag="py", bufs=4)
                    tr("pe", nc.tensor.matmul(mT_ps[:, :], mask01[:, :], ident16[:, :],
                                              is_transpose=True, start=True, stop=True))
                    tr("dve", nc.vector.tensor_copy(mask_sb[:, bass.ds(s * 128, 128)], mT_ps))

            # combine = sqrt(softplus * mask); sqrt last so the ACT table
            # order is exp/ln -> sqrt -> silu (2 switches total)
            for c in range(NCH):
                tsl = bass.ds(c * CT, CT)
                spm = rt.tile([E, CT], F32, tag="spm", bufs=2)
                tr("dve", nc.vector.tensor_mul(spm, sp_tiles[c], mask_sb[:, tsl]))
                tr("act", nc.scalar.activation(combine_sb[:, tsl], spm, AF.Sqrt))
                tr("dve", nc.vector.tensor_copy(combine16_sb[:, tsl], combine_sb[:, tsl]))

            # bounce combine through DRAM to broadcast rows across partitions
            dma(nc.sync, comb_dram[:, :], combine_sb[:, :])
            for e in range(E):
                dma(nc.sync, cbc_sb[:, e, :],
                    bass.AP(comb_dram.tensor, comb_dram.offset + e * NT, [[0, 128], [1, NT]]))

            # ---- expert MLPs --------------------------------------------------
            # y accumulated in SBUF across experts; shared expert (8) first.
            y_acc = []
            for c in range(NCH):
                y_acc_c = yp.tile([128, DT, CT], F32, tag=f"yacc{c}", bufs=1)
                y_acc.append(y_acc_c)
            eorder = [E] + list(range(E))
            for ei, e in enumerate(eorder):
                WQ = WCOLS // 4
                w_tiles = w_next
                if ei + 1 < NE:
                    w_next = load_w(eorder[ei + 1])
                last_e = ei == NE - 1

                def wcol(col):
                    return w_tiles[col // WQ][:, bass.ds(col % WQ, 128)]

                def w1t(k, j):
                    return wcol((k * HT + j) * 128)

                def w2t(j, d):
                    return wcol((DT * HT + j * DT + d) * 128)

                h1_tiles = {}
                for j in range(HT):
                    for c in range(NCH):
                        tsl = bass.ds(c * CT, CT)
                        h1 = h1p.tile([128, CT], BF16, tag=f"h1_{j}_{c}", bufs=3)
                        h1_tiles[(j, c)] = h1
                        ph = ps.tile([128, CT], F32, tag="ph", bufs=4)
                        for k in range(DT):
                            tr("pe", nc.tensor.matmul(
                                ph[:, :],
                                w1t(k, j),
                                xhi_t[k][:, tsl],
                                start=(k == 0), stop=(k == DT - 1),
                            ))
                        tr("act", nc.scalar.activation(
                            h1[:, :], ph, AF.Silu,
                            bias=b1_sb[:, e, j:j + 1], scale=1.0))
                        if e != E:
                            # in-place combine scale (keeps DVE as last writer)
                            tr("dve", nc.vector.tensor_mul(h1, h1, cbc_sb[:, e, tsl]))

                for c in range(NCH):
                    tsl = bass.ds(c * CT, CT)
                    for d in range(DT):
                        py = ps.tile([128, CT], F32, tag="py", bufs=4)
                        for j in range(HT):
                            tr("pe", nc.tensor.matmul(
                                py[:, :],
                                w2t(j, d),
                                h1_tiles[(j, c)][:, :],
                                start=(j == 0),
                                stop=(j == HT - 1 and not last_e),
                            ))
                        if last_e:
                            # rb2^T @ combine joins the last expert's group
                            tr("pe", nc.tensor.matmul(
                                py[:, :], rb2_sb[:, d, :], combine16_sb[:, tsl],
                                start=False, stop=True))
                        if ei == 0:
                            # shared expert first: y = py + sb2
                            tr("dve", nc.vector.tensor_scalar(
                                y_acc[c][:, d, :], py[:, :], b2_sb[:, d:d + 1], None,
                                op0=mybir.AluOpType.add))
                        else:
                            tr("dve", nc.vector.tensor_add(
                                y_acc[c][:, d, :], y_acc[c][:, d, :], py[:, :]))
                        if last_e:
                            # stream each output slice out as soon as its
                            # final accumulation lands
                            dma(nc.sync, yt[:, d, tsl], y_acc[c][:, d, :])

    nc.compile()
    return nc


_NC_CACHE = None


def _get_bass():
    global _NC_CACHE
    if _NC_CACHE is None:
        _NC_CACHE = _build_bass()
    return _NC_CACHE


def _pack_inputs(X, routing_W, expert_bias, sW1, sb1, sW2, sb2, rW1, rb1, rW2, rb2):
    bf = ml_dtypes.bfloat16
    f32 = np.float32

    Xf = np.ascontiguousarray(np.asarray(X, f32).reshape(B * T, D))

    w1 = np.concatenate([np.asarray(rW1, f32), np.asarray(sW1, f32)[None]], axis=0)
    w2 = np.concatenate([np.asarray(rW2, f32), np.asarray(sW2, f32)[None]], axis=0)
    b1 = np.concatenate([np.asarray(rb1, f32), np.asarray(sb1, f32)[None]], axis=0)

    w1h = w1.reshape(NE, DT, 128, HT, 128).transpose(0, 2, 1, 3, 4).reshape(
        NE, 128, DT * HT * 128)
    w2h = w2.reshape(NE, HT, 128, DT, 128).transpose(0, 2, 1, 3, 4).reshape(
        NE, 128, HT * DT * 128)
    wall = np.ascontiguousarray(np.concatenate([w1h, w2h], axis=2)).astype(bf)
    b1h = np.ascontiguousarray(b1.reshape(NE, HT, 128).transpose(2, 0, 1)).astype(f32)
    b2h = np.ascontiguousarray(np.asarray(sb2, f32).reshape(DT, 128).T).astype(f32)
    rb2h = np.ascontiguousarray(np.asarray(rb2, f32).reshape(E, DT, 128)).astype(bf)

    rwt = np.ascontiguousarray(np.asarray(routing_W, f32).T)       # [1024, 8]
    rwt_hi = rwt.astype(bf)
    rwt_lo = (rwt - rwt_hi.astype(f32)).astype(bf)
    rwh = np.ascontiguousarray(rwt_hi.reshape(DT, 128, E).transpose(1, 0, 2))
    rwl = np.ascontiguousarray(rwt_lo.reshape(DT, 128, E).transpose(1, 0, 2))

    eb = np.ascontiguousarray(np.asarray(expert_bias, f32).reshape(1, E))

    shared = dict(rwh=rwh, rwl=rwl, ebias=eb, wall=wall,
                  b1h=b1h, b2h=b2h, rb2h=rb2h)

    in_maps = []
    for c in range(N_CORES):
        Xs = np.ascontiguousarray(Xf[c * NT:(c + 1) * NT].T)  # [1024 d, 1024 t] f32
        xhi_a = Xs.astype(bf)
        xlo_a = (Xs - xhi_a.astype(f32)).astype(bf)
        xhi_p = np.ascontiguousarray(xhi_a.reshape(DT, 128, NT).transpose(1, 0, 2))
        xlo_p = np.ascontiguousarray(xlo_a.reshape(DT, 128, NT).transpose(1, 0, 2))
        in_maps.append(dict(xhi=xhi_p, xlo=xlo_p, **shared))
    return in_maps


def kernel(X, routing_W, expert_bias, sW1, sb1, sW2, sb2, rW1, rb1, rW2, rb2,
           _trace=False):
    in_maps = _pack_inputs(X, routing_W, expert_bias, sW1, sb1, sW2, sb2,
                           rW1, rb1, rW2, rb2)
    nc = _get_bass()
    res = run_bass_kernel_spmd(nc, in_maps, core_ids=list(range(N_CORES)),
                               trace=_trace)
    out = np.empty((B * T, D), np.float32)
    for c in range(N_CORES):
        ytc = res.results[c]["yt"]                       # [128, DT, NT]
        Yt = ytc.transpose(1, 0, 2).reshape(D, NT)       # [d, t]
        out[c * NT:(c + 1) * NT] = Yt.T
    out = out.reshape(B, T, D)
    if _trace:
        return out, res
    return out


# revision 27
# speedup vs baseline: 1.0299x; 1.0107x over previous
"""DeepSeekMoE Trainium2 kernel.

Strategy: data-parallel over tokens. Each of the 8 NeuronCores gets 1024 of
the 8192 tokens and computes routing + shared expert + all 8 routed experts
densely (unrouted experts contribute with combine weight 0, matching the
dense reference formulation exactly).

Layout: feature-major ("X^T") activations [d, tokens] so every weight matrix
is used in its natural [in, out] layout as the matmul stationary operand.

Numerics: expert MLPs run in bf16 (fp32 PSUM accumulation). Routing logits
are computed with a bf16 hi/lo split (X = hi + lo), giving ~fp32-accuracy
logits. Top-2 selection runs on the raw logits (sqrt(softplus) is monotonic,
so the selection is identical to the reference's); the combine weights are
then sqrt(ln(1+exp(logits)))*mask via the ACT exp/ln/sqrt tables.

Built with bacc.Bacc and nc.compile(), which legalizes semaphore waits
(1 wait/instruction on TRN2, split via EventSemaphores) and inserts the
ACT function-table loads (exp/ln -> sqrt -> silu, 2 switches). Weight
loads are 4-way split and prefetched one expert ahead; X slices are
independent tiles so consumers wait per-slice; outputs stream out as
their final accumulation lands.
"""

import numpy as np
import ml_dtypes

import concourse.bass as bass
import concourse.mybir as mybir
import concourse.tile as tile
from concourse import bacc
from concourse.tile_rust import add_dep_helper
from concourse.bass_utils import run_bass_kernel_spmd

BF16 = mybir.dt.bfloat16
F32 = mybir.dt.float32
AF = mybir.ActivationFunctionType

N_CORES = 8
B, T, D = 4, 2048, 1024
E = 8            # routed experts
H = 512          # expert hidden dim
NT = (B * T) // N_CORES   # tokens per core = 1024
DT = D // 128    # 8 d-tiles
HT = H // 128    # 4 h-tiles
NE = E + 1       # experts + shared (index 8 = shared)
CT = 512         # psum token chunk
NCH = NT // CT   # 2 chunks
WCOLS = (DT * HT + HT * DT) * 128  # 8192 weight columns per expert


def _build_bass():
    nc = bacc.Bacc("TRN2", num_devices=N_CORES, target_bir_lowering=False)

    xhi = nc.dram_tensor("xhi", [128, DT, NT], BF16, kind="ExternalInput")
    xlo = nc.dram_tensor("xlo", [128, DT, NT], BF16, kind="ExternalInput")
    rwh = nc.dram_tensor("rwh", [128, DT, E], BF16, kind="ExternalInput")
    rwl = nc.dram_tensor("rwl", [128, DT, E], BF16, kind="ExternalInput")
    ebias = nc.dram_tensor("ebias", [1, E], F32, kind="ExternalInput")
    wall = nc.dram_tensor("wall", [NE, 128, WCOLS], BF16, kind="ExternalInput")
    b1h = nc.dram_tensor("b1h", [128, NE, HT], F32, kind="ExternalInput")
    b2h = nc.dram_tensor("b2h", [128, DT], F32, kind="ExternalInput")
    rb2h = nc.dram_tensor("rb2h", [E, DT, 128], BF16, kind="ExternalInput")
    yt = nc.dram_tensor("yt", [128, DT, NT], F32, kind="ExternalOutput")

    all_dmas = []          # every dma_start instruction, in order
    last_on = {}           # engine name -> last instruction

    def dma(engine, out, in_):
        inst = engine.dma_start(out=out, in_=in_)
        all_dmas.append(inst)
        return inst

    def tr(name, inst):
        last_on[name] = inst
        return inst

    with tile.TileContext(nc) as tc:
        with (
            tc.tile_pool(name="singles", bufs=1) as singles,
            tc.tile_pool(name="wpool", bufs=2) as wpool,
            tc.tile_pool(name="rt", bufs=4) as rt,
            tc.tile_pool(name="h1p", bufs=2) as h1p,
            tc.tile_pool(name="yp", bufs=1) as yp,
            tc.tile_pool(name="ps", bufs=1, space="PSUM") as ps,
            tc.tile_pool(name="dram", bufs=1, space="DRAM") as dram,
        ):
            # ---- static loads -------------------------------------------------
            # issue order tracks the PE critical path: the k=0 X slice and
            # routing weights first (the first logit group), remaining X
            # slices next, descriptor-heavy constants last.
            xhi_t = [None] * DT
            xlo_t = [None] * DT

            def load_x(arr, dram_t, k, tag):
                x_k = singles.tile([128, NT], BF16, tag=tag)
                dma(nc.sync, x_k, dram_t[:, k, :])
                arr[k] = x_k

            load_x(xhi_t, xhi, 0, "xhi0")
            rwh_sb = singles.tile([128, DT, E], BF16)
            dma(nc.sync, rwh_sb, rwh[:, :, :])
            rwl_sb = singles.tile([128, DT, E], BF16)
            dma(nc.sync, rwl_sb, rwl[:, :, :])
            for k in range(1, DT):
                load_x(xhi_t, xhi, k, f"xhi{k}")
            for k in range(DT):
                load_x(xlo_t, xlo, k, f"xlo{k}")

            eb_sb = singles.tile([E, 1], F32)
            dma(nc.sync, eb_sb, bass.AP(ebias, 0, [[1, E], [0, 1]]))
            b1_sb = singles.tile([128, NE, HT], F32)
            dma(nc.sync, b1_sb, b1h[:, :, :])
            b2_sb = singles.tile([128, DT], F32)
            dma(nc.sync, b2_sb, b2h[:, :])
            rb2_sb = singles.tile([E, DT, 128], BF16)
            dma(nc.sync, rb2_sb, rb2h[:, :, :])

            wdmas = []

            def load_w(e):
                tiles = []
                for q in range(4):
                    w_q = wpool.tile([128, WCOLS // 4], BF16, tag=f"w{q}", bufs=2)
                    tiles.append(w_q)
                    wdmas.append(dma(nc.sync, w_q,
                                     wall[e, :, bass.ds(q * (WCOLS // 4), WCOLS // 4)]))
                return tiles

            w_next = load_w(E)  # shared expert's weights prefetch

            ident16 = singles.tile([128, 128], BF16)
            nc.gpsimd.memset(ident16, 0.0)
            tr("pool", nc.gpsimd.affine_select(
                out=ident16, in_=ident16, compare_op=mybir.AluOpType.not_equal,
                fill=1.0, base=0, pattern=[[-1, 128]], channel_multiplier=1))
            ident8b = singles.tile([8, 8], BF16)
            nc.gpsimd.memset(ident8b, 0.0)
            tr("pool", nc.gpsimd.affine_select(
                out=ident8b, in_=ident8b, compare_op=mybir.AluOpType.not_equal,
                fill=1.0, base=0, pattern=[[-1, 8]], channel_multiplier=1))

            # warm-up: have DVE observe the small constant tiles' DMA queues
            # so later 1-wait-capped TensorScalarPtr ops don't need them
            warm = singles.tile([128, 8], F32)
            tr("dve", nc.vector.tensor_copy(warm[0:E, 0:1], eb_sb))
            tr("dve", nc.vector.tensor_copy(warm[:, 1:2], b2_sb[:, 0:1]))
            tr("dve", nc.vector.tensor_copy(warm[:, 2:3], b1_sb[:, 0, 0:1]))

            combine_sb = singles.tile([E, NT], F32)
            combine16_sb = singles.tile([E, NT], BF16)
            mask_sb = singles.tile([E, NT], BF16)
            cbc_sb = singles.tile([128, E, NT], F32)
            comb_dram = dram.tile([E, NT], F32)

            # ---- routing ------------------------------------------------------
            # logits feature-major [E, t] = sum_k rwt[dk].T @ Xt[dk]; bf16 hi/lo
            sp_tiles = []
            lg_tiles = []
            for c in range(NCH):
                lg_c = ps.tile([E, CT], F32, tag="ph", bufs=4)
                lg_tiles.append(lg_c)
            # product-major emission: both chunks' hi-terms first so the PE
            # doesn't stall mid-group waiting for the xlo DMAs
            prev_mm = None
            for pi, (xa, wa) in enumerate(
                    ((xhi_t, rwh_sb), (xhi_t, rwl_sb), (xlo_t, rwh_sb))):
                for c in range(NCH):
                    for k in range(DT):
                        mm = tr("pe", nc.tensor.matmul(
                            lg_tiles[c][:, :],
                            wa[:, k, :],
                            xa[k][:, bass.ds(c * CT, CT)],
                            start=(pi == 0 and k == 0),
                            stop=(pi == 2 and k == DT - 1),
                            skip_group_check=True,
                        ))
                        if prev_mm is not None:
                            # keep product-major order through the scheduler so
                            # the xlo-dependent terms issue last
                            add_dep_helper(mm.ins, prev_mm.ins, False, "mm order")
                        prev_mm = mm
            for c in range(NCH):
                tsl = bass.ds(c * CT, CT)
                lg_ps = lg_tiles[c]
                # logits + expert_bias -> sbuf (fp32)
                lgf = rt.tile([E, CT], F32, tag="lgf", bufs=2)
                tr("dve", nc.vector.tensor_scalar(
                    lgf, lg_ps, eb_sb[:, :], None, op0=mybir.AluOpType.add))
                # hi/lo split for exact bf16 transposes
                lghi = rt.tile([E, CT], BF16, tag="lghi", bufs=2)
                tr("dve", nc.vector.tensor_copy(lghi, lgf))
                lglo32 = rt.tile([E, CT], F32, tag="lglo32", bufs=2)
                tr("dve", nc.vector.tensor_sub(lglo32, lgf, lghi))
                lglo = rt.tile([E, CT], BF16, tag="lglo", bufs=2)
                tr("dve", nc.vector.tensor_copy(lglo, lglo32))
                # softplus (exp then ln) on ACT, feature-major
                u_t = rt.tile([E, CT], F32, tag="u", bufs=2)
                tr("act", nc.scalar.activation(u_t, lgf, AF.Exp))
                sp_t = rt.tile([E, CT], F32, tag="sp", bufs=2)
                tr("act", nc.scalar.activation(sp_t, u_t, AF.Ln, bias=1.0, scale=1.0))
                sp_tiles.append(sp_t)

                for s4 in range(CT // 128):
                    s = c * (CT // 128) + s4
                    csl = bass.ds(s4 * 128, 128)
                    trh_ps = ps.tile([128, E], BF16, tag="py", bufs=4)
                    tr("pe", nc.tensor.matmul(trh_ps[:, :], lghi[:, csl], ident8b[:, :],
                                              is_transpose=True, start=True, stop=True))
                    trl_ps = ps.tile([128, E], BF16, tag="py", bufs=4)
                    tr("pe", nc.tensor.matmul(trl_ps[:, :], lglo[:, csl], ident8b[:, :],
                                              is_transpose=True, start=True, stop=True))
                    trh_sb = rt.tile([128, E], F32, tag="trh")
                    tr("dve", nc.vector.tensor_copy(trh_sb, trh_ps))
                    lg_tok = rt.tile([128, E], F32, tag="lgtok")
                    tr("dve", nc.vector.tensor_add(lg_tok, trl_ps, trh_sb))
                    # top-2 selection on logits (monotonic in the activation)
                    m1 = rt.tile([128, 1], F32, tag="m1")
                    tr("dve", nc.vector.tensor_reduce(m1, lg_tok, axis=mybir.AxisListType.X, op=mybir.AluOpType.max))
                    msk = rt.tile([128, E], F32, tag="msk")
                    tr("dve", nc.vector.tensor_scalar(msk, lg_tok, m1[:, :], None, op0=mybir.AluOpType.is_ge))
                    masked = rt.tile([128, E], F32, tag="mskd")
                    tr("dve", nc.vector.scalar_tensor_tensor(
                        masked, msk, -1e30, lg_tok,
                        op0=mybir.AluOpType.mult, op1=mybir.AluOpType.add))
                    m2 = rt.tile([128, 1], F32, tag="m2")
                    tr("dve", nc.vector.tensor_reduce(m2, masked, axis=mybir.AxisListType.X, op=mybir.AluOpType.max))
                    mask01 = rt.tile([128, E], BF16, tag="mask01")
                    tr("dve", nc.vector.tensor_scalar(mask01, lg_tok, m2[:, :], None, op0=mybir.AluOpType.is_ge))
                    # mask back to feature-major (0/1 exact in bf16)
                    mT_ps = ps.tile([E, 128], BF16, tag="py", bufs=4)
                    tr("pe", nc.tensor.matmul(mT_ps[:, :], mask01[:, :], ident16[:, :],
                                              is_transpose=True, start=True, stop=True))
                    tr("dve", nc.vector.tensor_copy(mask_sb[:, bass.ds(s * 128, 128)], mT_ps))

            # combine = sqrt(softplus * mask); sqrt last so the ACT table
            # order is exp/ln -> sqrt -> silu (2 switches total)
            for c in range(NCH):
                tsl = bass.ds(c * CT, CT)
                spm = rt.tile([E, CT], F32, tag="spm", bufs=2)
                tr("dve", nc.vector.tensor_mul(spm, sp_tiles[c], mask_sb[:, tsl]))
                tr("act", nc.scalar.activation(combine_sb[:, tsl], spm, AF.Sqrt))
                tr("dve", nc.vector.tensor_copy(combine16_sb[:, tsl], combine_sb[:, tsl]))

            # bounce combine through DRAM to broadcast rows across partitions
            dma(nc.sync, comb_dram[:, :], combine_sb[:, :])
            for e in range(E):
                dma(nc.sync, cbc_sb[:, e, :],
                    bass.AP(comb_dram.tensor, comb_dram.offset + e * NT, [[0, 128], [1, NT]]))

            # ---- expert MLPs --------------------------------------------------
            # y accumulated in SBUF across experts; shared expert (8) first.
            y_acc = []
            for c in range(NCH):
                y_acc_c = yp.tile([128, DT, CT], F32, tag=f"yacc{c}", bufs=1)
                y_acc.append(y_acc_c)
            eorder = [E] + list(range(E))
            for ei, e in enumerate(eorder):
                WQ = WCOLS // 4
                w_tiles = w_next
                if ei + 1 < NE:
                    w_next = load_w(eorder[ei + 1])
                last_e = ei == NE - 1

                def wcol(col):
                    return w_tiles[col // WQ][:, bass.ds(col % WQ, 128)]

                def w1t(k, j):
                    return wcol((k * HT + j) * 128)

                def w2t(j, d):
                    return wcol((DT * HT + j * DT + d) * 128)

                h1_tiles = {}
                for j in range(HT):
                    for c in range(NCH):
                        tsl = bass.ds(c * CT, CT)
                        h1 = h1p.tile([128, CT], BF16, tag=f"h1_{j}_{c}", bufs=3)
                        h1_tiles[(j, c)] = h1
                        ph = ps.tile([128, CT], F32, tag="ph", bufs=4)
                        for k in range(DT):
                            tr("pe", nc.tensor.matmul(
                                ph[:, :],
                                w1t(k, j),
                                xhi_t[k][:, tsl],
                                start=(k == 0), stop=(k == DT - 1),
                            ))
                        tr("act", nc.scalar.activation(
                            h1[:, :], ph, AF.Silu,
                            bias=b1_sb[:, e, j:j + 1], scale=1.0))
                        if e != E:
                            # in-place combine scale (keeps DVE as last writer)
                            tr("dve", nc.vector.tensor_mul(h1, h1, cbc_sb[:, e, tsl]))

                for c in range(NCH):
                    tsl = bass.ds(c * CT, CT)
                    for d in range(DT):
                        py = ps.tile([128, CT], F32, tag="py", bufs=4)
                        for j in range(HT):
                            tr("pe", nc.tensor.matmul(
                                py[:, :],
                                w2t(j, d),
                                h1_tiles[(j, c)][:, :],
                                start=(j == 0),
                                stop=(j == HT - 1 and not last_e),
                            ))
                        if last_e:
                            # rb2^T @ combine joins the last expert's group
                            tr("pe", nc.tensor.matmul(
                                py[:, :], rb2_sb[:, d, :], combine16_sb[:, tsl],
                                start=False, stop=True))
                        if ei == 0:
                            # shared expert first: y = py + sb2
                            tr("dve", nc.vector.tensor_scalar(
                                y_acc[c][:, d, :], py[:, :], b2_sb[:, d:d + 1], None,
                                op0=mybir.AluOpType.add))
                        else:
                            tr("dve", nc.vector.tensor_add(
                                y_acc[c][:, d, :], y_acc[c][:, d, :], py[:, :]))
                        if last_e:
                            # stream each output slice out as soon as its
                            # final accumulation lands
                            dma(nc.sync, yt[:, d, tsl], y_acc[c][:, d, :])

    nc.compile()
    return nc


_NC_CACHE = None


def _get_bass():
    global _NC_CACHE
    if _NC_CACHE is None:
        _NC_CACHE = _build_bass()
    return _NC_CACHE


def _pack_inputs(X, routing_W, expert_bias, sW1, sb1, sW2, sb2, rW1, rb1, rW2, rb2):
    bf = ml_dtypes.bfloat16
    f32 = np.float32

    Xf = np.ascontiguousarray(np.asarray(X, f32).reshape(B * T, D))

    w1 = np.concatenate([np.asarray(rW1, f32), np.asarray(sW1, f32)[None]], axis=0)
    w2 = np.concatenate([np.asarray(rW2, f32), np.asarray(sW2, f32)[None]], axis=0)
    b1 = np.concatenate([np.asarray(rb1, f32), np.asarray(sb1, f32)[None]], axis=0)

    w1h = w1.reshape(NE, DT, 128, HT, 128).transpose(0, 2, 1, 3, 4).reshape(
        NE, 128, DT * HT * 128)
    w2h = w2.reshape(NE, HT, 128, DT, 128).transpose(0, 2, 1, 3, 4).reshape(
        NE, 128, HT * DT * 128)
    wall = np.ascontiguousarray(np.concatenate([w1h, w2h], axis=2)).astype(bf)
    b1h = np.ascontiguousarray(b1.reshape(NE, HT, 128).transpose(2, 0, 1)).astype(f32)
    b2h = np.ascontiguousarray(np.asarray(sb2, f32).reshape(DT, 128).T).astype(f32)
    rb2h = np.ascontiguousarray(np.asarray(rb2, f32).reshape(E, DT, 128)).astype(bf)

    rwt = np.ascontiguousarray(np.asarray(routing_W, f32).T)       # [1024, 8]
    rwt_hi = rwt.astype(bf)
    rwt_lo = (rwt - rwt_hi.astype(f32)).astype(bf)
    rwh = np.ascontiguousarray(rwt_hi.reshape(DT, 128, E).transpose(1, 0, 2))
    rwl = np.ascontiguousarray(rwt_lo.reshape(DT, 128, E).transpose(1, 0, 2))

    eb = np.ascontiguousarray(np.asarray(expert_bias, f32).reshape(1, E))

    shared = dict(rwh=rwh, rwl=rwl, ebias=eb, wall=wall,
                  b1h=b1h, b2h=b2h, rb2h=rb2h)

    in_maps = []
    for c in range(N_CORES):
        Xs = np.ascontiguousarray(Xf[c * NT:(c + 1) * NT].T)  # [1024 d, 1024 t] f32
        xhi_a = Xs.astype(bf)
        xlo_a = (Xs - xhi_a.astype(f32)).astype(bf)
        xhi_p = np.ascontiguousarray(xhi_a.reshape(DT, 128, NT).transpose(1, 0, 2))
        xlo_p = np.ascontiguousarray(xlo_a.reshape(DT, 128, NT).transpose(1, 0, 2))
        in_maps.append(dict(xhi=xhi_p, xlo=xlo_p, **shared))
    return in_maps


def kernel(X, routing_W, expert_bias, sW1, sb1, sW2, sb2, rW1, rb1, rW2, rb2,
           _trace=False):
    in_maps = _pack_inputs(X, routing_W, expert_bias, sW1, sb1, sW2, sb2,
                           rW1, rb1, rW2, rb2)
    nc = _get_bass()
    res = run_bass_kernel_spmd(nc, in_maps, core_ids=list(range(N_CORES)),
                               trace=_trace)
    out = np.empty((B * T, D), np.float32)
    for c in range(N_CORES):
        ytc = res.results[c]["yt"]                       # [128, DT, NT]
        Yt = ytc.transpose(1, 0, 2).reshape(D, NT)       # [d, t]
        out[c * NT:(c + 1) * NT] = Yt.T
    out = out.reshape(B, T, D)
    if _trace:
        return out, res
    return out
